# revision 23
# baseline (speedup 1.0000x reference)
"""Multi-head attention (B=4, S=2048, D=1024, H=16) on 8 trn2 NeuronCores.

Sharding: core c -> (batch b = c//2, head-group hg = c%2 of 8 heads).
Each core computes q/k/v projections for its 8 heads, attention, and a
partial output projection (its heads' contribution). Host sums the two
partials per batch and adds b_O.

Pipeline (the attention phase is ACT(exp)-bound at ~1.12us/kt, so all
other work rides in the PE/DVE/GPSIMD slack of the kt loop):
  prologue: vhat (X stationary, ones column -> softmax Z falls out of
    PV), full k-projection, qT[0]; X/W DMAs spread over three queues.
  attention blocks (hp outer, qc inner): per kt -> scoresT (two heads
    on concurrent PE quadrants, K=64), ACT exp -> PT, PV accumulate.
    Deferred into fixed kt slots of the next block: Z-gather + recip
    (DVE), 1/Z partition-broadcast (GPSIMD), normalize-mul (DVE,
    writes attn_outT in place of qT), output-projection pieces (PE +
    DVE accumulate), and the q-projection of head-pair hp+1 (PE, one
    matmul per even slot, accumulating in a borrowed PSUM bank).
"""
import sys

if '/opt/trn_rl_repo' not in sys.path:
    sys.path.insert(0, '/opt/trn_rl_repo')

import ml_dtypes
import numpy as np

import concourse.bass as bass
import concourse.tile as tile
from concourse import bacc, mybir
from concourse.bass_utils import run_bass_kernel_spmd

N_CORES = 8
B, S, D = 4, 2048, 1024
H = 16
DH = 64                 # head dim
HC = 8                  # heads per core
C = HC * DH             # per-core projection width = 512
F32 = mybir.dt.float32
F32R = mybir.dt.float32r
BF16 = mybir.dt.bfloat16

NKT = S // 128          # 16 s-tiles of 128
NM = C // 128           # 4 c-tiles (head pairs)
NDK = D // 128          # 8 contraction tiles for projections
SCALE = 1.0 / np.sqrt(DH)

PROJ_DT = BF16
QK_DT = BF16
PV_DT = BF16
OUT_DT = BF16


def round_fp32r(x):
    b = np.ascontiguousarray(x, dtype=np.float32).view(np.uint32)
    b = (b + 0x800) & np.uint32(0xFFFFF000)
    return b.view(np.float32)


def prep(x, dt):
    if dt == BF16:
        return np.ascontiguousarray(x).astype(ml_dtypes.bfloat16)
    return round_fp32r(x)


def build():
    nc = bacc.Bacc("TRN2", target_bir_lowering=False, debug=False,
                   num_devices=N_CORES)
    XqT = nc.dram_tensor("XqT", [D, S], PROJ_DT, kind="ExternalInput").ap()
    XkT = nc.dram_tensor("XkT", [D, S], PROJ_DT, kind="ExternalInput").ap()
    XvT = nc.dram_tensor("XvT", [D, S], PROJ_DT, kind="ExternalInput").ap()
    Wq = nc.dram_tensor("Wq", [D, C], PROJ_DT, kind="ExternalInput").ap()
    Wk = nc.dram_tensor("Wk", [D, C], PROJ_DT, kind="ExternalInput").ap()
    Wv = nc.dram_tensor("Wv", [D, C], PROJ_DT, kind="ExternalInput").ap()
    Wo = nc.dram_tensor("Wo", [C, D], OUT_DT, kind="ExternalInput").ap()
    bq = nc.dram_tensor("bq", [C], F32, kind="ExternalInput").ap()
    bk = nc.dram_tensor("bk", [C], F32, kind="ExternalInput").ap()
    bv = nc.dram_tensor("bv", [C], F32, kind="ExternalInput").ap()
    OP = nc.dram_tensor("OP", [S, D], OUT_DT, kind="ExternalOutput").ap()

    with tile.TileContext(nc) as tc:
        _build_body(nc, tc, XqT, XkT, XvT, Wq, Wk, Wv, Wo, bq, bk, bv, OP)
    nc.compile()
    return nc


def _build_body(nc, tc, XqT, XkT, XvT, Wq, Wk, Wv, Wo, bq, bk, bv, OP):
    from contextlib import ExitStack
    with ExitStack() as stack:
        consts = stack.enter_context(tc.tile_pool(name="consts", bufs=1))
        qkp = stack.enter_context(tc.tile_pool(name="qk", bufs=2 * NM))
        vhp = stack.enter_context(tc.tile_pool(name="vh", bufs=NKT))
        xqp = stack.enter_context(tc.tile_pool(name="xq", bufs=NDK))
        wqp = stack.enter_context(tc.tile_pool(name="wq", bufs=NDK))
        wop = stack.enter_context(tc.tile_pool(name="wo", bufs=NM))

        # constants
        ones_f32 = consts.tile([128, 1], F32)
        nc.vector.memset(ones_f32, 1.0)
        # persistent Z / 1/Z tiles: both heads' Z rows packed at partitions
        # 0 and 32 so a single reciprocal covers both (per-lane serial, so
        # [33,512] costs the same as [1,512]). memset once so the unused
        # rows are never read uninitialized.
        zt_t = consts.tile([33, 512], BF16)
        rz_t = consts.tile([33, 512], BF16)
        nc.vector.memset(zt_t, 1.0)
        bias_t = consts.tile([128, 2 * NM], F32)
        for i, b_ in enumerate((bq, bk)):
            nc.sync.dma_start(
                out=bias_t[:, i * NM:(i + 1) * NM],
                in_=b_.rearrange("(m p) -> p m", p=128))
        bvb = consts.tile([128, C], F32)
        nc.gpsimd.dma_start(
            out=bvb,
            in_=bass.AP(tensor=bv.tensor, offset=0, ap=[[0, 128], [1, C]]))

        qT = [None] * NM
        kT = [None] * NM
        vhat = [None] * NKT
        for m in range(NM):
            qT[m] = qkp.tile([128, S], QK_DT, tag="qk", name=f"qTt{m}")
            kT[m] = qkp.tile([128, S], QK_DT, tag="qk", name=f"kTt{m}")
        # attn_outT aliases qT: by the time block (hp,qc)'s normalize-mul
        # writes qT[hp][:, qc*512:...], no scores matmul reads those
        # columns again (single pass, hp outer).
        attn_outT = qT

        # ---------------- prologue: vhat, full k-proj, qT[0] -------------
        with ExitStack() as pstack:
            xtp = pstack.enter_context(tc.tile_pool(name="xt", bufs=16))
            wp = pstack.enter_context(tc.tile_pool(name="w", bufs=16))
            pjp = pstack.enter_context(
                tc.tile_pool(name="pj", bufs=3, space="PSUM"))

            # v: X stationary [128,128] slices, Wv moving; ones col packed
            xv = []
            wv = []
            for kt in range(NDK):
                x = xtp.tile([128, S], PROJ_DT, tag="xt", name=f"xv{kt}")
                eng = nc.sync if kt % 2 == 0 else nc.scalar
                eng.dma_start(out=x, in_=XvT[kt * 128:(kt + 1) * 128, :])
                xv.append(x)
                w = wp.tile([128, C], PROJ_DT, tag="w", name=f"wv{kt}")
                nc.gpsimd.dma_start(out=w, in_=Wv[kt * 128:(kt + 1) * 128, :])
                wv.append(w)
            # kick off k DMAs right behind v on the same queues
            xk = []
            wk = []
            for kt in range(NDK):
                x = xtp.tile([128, S], PROJ_DT, tag="xt", name=f"xk{kt}")
                eng = nc.sync if kt % 2 == 0 else nc.scalar
                eng.dma_start(out=x, in_=XkT[kt * 128:(kt + 1) * 128, :])
                xk.append(x)
                w = wp.tile([128, C], PROJ_DT, tag="w", name=f"wk{kt}")
                nc.gpsimd.dma_start(out=w, in_=Wk[kt * 128:(kt + 1) * 128, :])
                wk.append(w)

            # Xq / Wq / Wo DMAs queued AFTER v and k so the prologue compute
            # is never starved; all stay resident for the interleaved q-proj
            # and output projection.
            xq = []
            wq = []
            for kt in range(NDK):
                x = xqp.tile([128, S], PROJ_DT, tag="xq", name=f"xq{kt}")
                eng = nc.sync if kt % 2 == 0 else nc.scalar
                eng.dma_start(out=x, in_=XqT[kt * 128:(kt + 1) * 128, :])
                xq.append(x)
                w = wqp.tile([128, C], PROJ_DT, tag="wq", name=f"wq{kt}")
                nc.gpsimd.dma_start(out=w, in_=Wq[kt * 128:(kt + 1) * 128, :])
                wq.append(w)
            wo_tiles = []
            for m in range(NM):
                w = wop.tile([128, D], OUT_DT, tag="wo", name=f"wo{m}")
                nc.gpsimd.dma_start(out=w, in_=Wo[m * 128:(m + 1) * 128, :])
                wo_tiles.append(w)

            for st in range(NKT):
                ps = pjp.tile([128, C], F32, tag="pj", name=f"vps{st}")
                for kt in range(NDK):
                    nc.tensor.matmul(
                        ps,
                        xv[kt][:, st * 128:(st + 1) * 128],
                        wv[kt],
                        start=(kt == 0), stop=(kt == NDK - 1))
                vh = vhp.tile([128, HC, DH + 1], PV_DT, tag="vh",
                              name=f"vhat{st}")
                with nc.allow_low_precision(reason="v epilogue"):
                    nc.vector.tensor_add(
                        vh[:, :, 0:DH],
                        ps.rearrange("p (h d) -> p h d", h=HC),
                        bvb.rearrange("p (h d) -> p h d", h=HC))
                    nc.vector.tensor_copy(
                        vh[:, :, DH], ones_f32.broadcast_to((128, HC)))
                vhat[st] = vh

            # full k projection (epilogues on ACT: it is idle here)
            for m in range(NM):
                for qtr in range(4):
                    ps = pjp.tile([128, 512], F32, tag="pj")
                    for kt in range(NDK):
                        nc.tensor.matmul(
                            ps,
                            wk[kt][:, m * 128:(m + 1) * 128],
                            xk[kt][:, qtr * 512:(qtr + 1) * 512],
                            start=(kt == 0), stop=(kt == NDK - 1))
                    with nc.allow_low_precision(reason="k epi"):
                        nc.scalar.activation(
                            out=kT[m][:, qtr * 512:(qtr + 1) * 512], in_=ps,
                            func=mybir.ActivationFunctionType.Identity,
                            bias=bias_t[:, NM + m:NM + m + 1], scale=1.0)

            # qT[0] (resident xq/wq)
            for qtr in range(4):
                ps = pjp.tile([128, 512], F32, tag="pj")
                for kt in range(NDK):
                    nc.tensor.matmul(
                        ps,
                        wq[kt][:, 0:128],
                        xq[kt][:, qtr * 512:(qtr + 1) * 512],
                        start=(kt == 0), stop=(kt == NDK - 1))
                with nc.allow_low_precision(reason="q epi"):
                    nc.scalar.activation(
                        out=qT[0][:, qtr * 512:(qtr + 1) * 512], in_=ps,
                        func=mybir.ActivationFunctionType.Identity,
                        bias=bias_t[:, 0:1], scale=1.0)

        # ---------------- attention + interleaved q-proj/out-proj --------
        with ExitStack() as astack:
            ptp = astack.enter_context(tc.tile_pool(name="pt", bufs=6))
            stg = astack.enter_context(tc.tile_pool(name="stg", bufs=4))
            nrm = astack.enter_context(tc.tile_pool(name="nrm", bufs=8))
            oap = astack.enter_context(tc.tile_pool(name="oacc", bufs=32))
            sp = astack.enter_context(
                tc.tile_pool(name="sps", bufs=2, space="PSUM"))
            pvp = astack.enter_context(
                tc.tile_pool(name="pv", bufs=2, space="PSUM"))
            opp = astack.enter_context(
                tc.tile_pool(name="op", bufs=1, space="PSUM"))
            qpp = astack.enter_context(
                tc.tile_pool(name="qp", bufs=1, space="PSUM"))

            out_acc = [[None] * 2 for _ in range(NKT)]

            def outproj_piece(hp, st, oc):
                ps = opp.tile([128, 512], F32, tag="op",
                              name=f"ops{hp}_{st}_{oc}")
                nc.tensor.matmul(
                    ps,
                    attn_outT[hp][:, st * 128:(st + 1) * 128],
                    wo_tiles[hp][:, oc * 512:(oc + 1) * 512],
                    start=True, stop=True)
                if hp == 0:
                    oa = oap.tile([128, 512], OUT_DT, tag="oacc",
                                  name=f"oacc{st}_{oc}")
                    out_acc[st][oc] = oa
                    with nc.allow_low_precision(reason="oacc bf16"):
                        nc.vector.tensor_copy(oa, ps)
                else:
                    oa = out_acc[st][oc]
                    with nc.allow_low_precision(reason="oacc bf16"):
                        nc.vector.tensor_add(oa, oa, ps)
                if hp == NM - 1:
                    eng = nc.sync if oc == 0 else nc.gpsimd
                    eng.dma_start(
                        out=OP[st * 128:(st + 1) * 128,
                               oc * 512:(oc + 1) * 512],
                        in_=oa)

            # Deferred pieces: normalize chain (recip/bcast/mul) runs one
            # block later; output-projection pieces run TWO blocks later so
            # the slow reciprocal in the DVE FIFO never delays the PSUM-bank
            # rotation of the outproj pieces.
            dnorm = []          # [(slot, fn)] for the next block
            dproj_1 = []        # outproj pieces, fire in next block
            dproj_2 = []        # outproj pieces, fire in the block after

            def make_norm_pieces(hp, qc, pvA, pvB):
                q0 = qc * 512
                sts = [None, None]
                bcs = [None, None]

                def stage(hh):
                    acc = pvA if hh == 0 else pvB
                    st_t = stg.tile([DH + 1, 512], F32, tag="stg",
                                    name=f"stg{hp}_{qc}_{hh}")
                    nc.vector.tensor_copy(st_t, acc)
                    sts[hh] = st_t

                def copyz(hh):
                    with nc.allow_low_precision(reason="Z bf16"):
                        nc.vector.tensor_copy(
                            zt_t[32 * hh:32 * hh + 1, :],
                            sts[hh][DH:DH + 1, :])

                rzb_t = [None]

                def recip():
                    with nc.allow_low_precision(reason="recip bf16"):
                        nc.vector.reciprocal(out=rz_t, in_=zt_t)
                    # stage head B's 1/Z to partition 0 right away
                    # (partition_broadcast ucode reads partition 0 only)
                    rzb = nrm.tile([1, 512], BF16, tag="rzb",
                                   name=f"rzb{hp}_{qc}")
                    nc.vector.tensor_copy(rzb, rz_t[32:33, :])
                    rzb_t[0] = rzb

                def bcast(hh):
                    src_ap = rz_t[0:1, :] if hh == 0 else rzb_t[0]
                    bc = nrm.tile([DH, 512], BF16, tag="bc",
                                  name=f"bcb{hp}_{qc}_{hh}")
                    nc.gpsimd.partition_broadcast(bc, src_ap)
                    bcs[hh] = bc

                def mul(hh):
                    # on GPSIMD (all-SBUF) to keep the DVE FIFO clear for
                    # the outproj PSUM-bank rotation
                    dlo = hh * DH
                    with nc.allow_low_precision(reason="attn_outT"):
                        nc.gpsimd.tensor_mul(
                            attn_outT[hp][dlo:dlo + DH, q0:q0 + 512],
                            sts[hh][0:DH, :], bcs[hh])

                # run NOW: free the PV PSUM banks, snapshot Z
                stage(0)
                stage(1)
                copyz(0)
                copyz(1)
                return [(8, recip), (9, lambda: bcast(0)),
                        (10, lambda: bcast(1)), (11, lambda: mul(0)),
                        (12, lambda: mul(1))]

            def make_outproj_pieces(hp, qc):
                return [(2 * j + 1, (lambda st, oc: lambda: outproj_piece(
                            hp, st, oc))(qc * 4 + j // 2, j % 2))
                        for j in range(8)]

            for hp in range(NM):
                for qc in range(4):
                    q0 = qc * 512
                    pvA = pvp.tile([DH + 1, 512], F32, tag="pv",
                                   name=f"pvA{hp}_{qc}")
                    pvB = pvp.tile([DH + 1, 512], F32, tag="pv",
                                   name=f"pvB{hp}_{qc}")
                    slotmap = {}
                    for s, fn in dnorm + dproj_2:
                        slotmap.setdefault(s, []).append(fn)
                    dnorm = []
                    dproj_2 = dproj_1
                    dproj_1 = []
                    # q-projection quarter for head-pair hp+1: one matmul
                    # per odd kt slot into its own PSUM bank.
                    pm = hp + 1 if hp < NM - 1 else None
                    qps = None
                    if pm is not None:
                        qps = qpp.tile([128, 512], F32, tag="qp",
                                       name=f"qps{pm}_{qc}")
                    for kt in range(NKT):
                        sps = sp.tile([128, 1024], F32, tag="sps")
                        for hh in range(2):
                            dlo = hh * DH
                            nc.tensor.matmul(
                                sps[:, hh * 512:(hh + 1) * 512],
                                kT[hp][dlo:dlo + DH,
                                       kt * 128:(kt + 1) * 128],
                                qT[hp][dlo:dlo + DH, q0:q0 + 512],
                                start=True, stop=True)
                        pt = ptp.tile([128, 1024], PV_DT, tag="pt")
                        nc.scalar.activation(
                            out=pt, in_=sps,
                            func=mybir.ActivationFunctionType.Exp,
                            scale=float(SCALE))
                        nc.tensor.matmul(
                            pvA, vhat[kt][:, 2 * hp, :], pt[:, 0:512],
                            start=(kt == 0), stop=(kt == NKT - 1))
                        nc.tensor.matmul(
                            pvB, vhat[kt][:, 2 * hp + 1, :], pt[:, 512:1024],
                            start=(kt == 0), stop=(kt == NKT - 1))
                        if pm is not None and kt % 2 == 0:
                            pkt = kt // 2
                            nc.tensor.matmul(
                                qps,
                                wq[pkt][:, pm * 128:(pm + 1) * 128],
                                xq[pkt][:, q0:q0 + 512],
                                start=(pkt == 0), stop=(pkt == NDK - 1))
                        for fn in slotmap.get(kt, ()):
                            fn()
                        if pm is not None and kt == 15:
                            with nc.allow_low_precision(reason="q epi dve"):
                                nc.vector.tensor_add(
                                    qT[pm][:, q0:q0 + 512], qps,
                                    bias_t[:, pm:pm + 1].broadcast_to(
                                        (128, 512)))
                    dnorm = make_norm_pieces(hp, qc, pvA, pvB)
                    dproj_1 = make_outproj_pieces(hp, qc)

            # flush the remaining pieces: start the normalize chain first,
            # overlap the (3,2) outproj pieces with it, then (3,3)
            for _, fn in dnorm + dproj_2:
                fn()
            for _, fn in dproj_1:
                fn()


_NC_CACHE = None
_last_in_maps = None


def _get_nc():
    global _NC_CACHE
    if _NC_CACHE is None:
        _NC_CACHE = build()
    return _NC_CACHE


def kernel(Q, K, V, W_Q, b_Q, W_K, b_K, W_V, b_V, W_O, b_O):
    global _last_in_maps
    Q = np.asarray(Q, dtype=np.float32)
    K = np.asarray(K, dtype=np.float32)
    V = np.asarray(V, dtype=np.float32)
    nc = _get_nc()

    XqTs = [prep(Q[b].T, PROJ_DT) for b in range(B)]
    XkTs = [prep(K[b].T, PROJ_DT) for b in range(B)]
    XvTs = [prep(V[b].T, PROJ_DT) for b in range(B)]
    Wqs = [prep(np.asarray(W_Q)[:, hg * C:(hg + 1) * C], PROJ_DT)
           for hg in range(2)]
    Wks = [prep(np.asarray(W_K)[:, hg * C:(hg + 1) * C], PROJ_DT)
           for hg in range(2)]
    Wvs = [prep(np.asarray(W_V)[:, hg * C:(hg + 1) * C], PROJ_DT)
           for hg in range(2)]
    Wos = [prep(np.asarray(W_O)[hg * C:(hg + 1) * C, :], OUT_DT)
           for hg in range(2)]
    bqs = [np.ascontiguousarray(np.asarray(b_Q, dtype=np.float32)[hg * C:(hg + 1) * C])
           for hg in range(2)]
    bks = [np.ascontiguousarray(np.asarray(b_K, dtype=np.float32)[hg * C:(hg + 1) * C])
           for hg in range(2)]
    bvs = [np.ascontiguousarray(np.asarray(b_V, dtype=np.float32)[hg * C:(hg + 1) * C])
           for hg in range(2)]

    in_maps = []
    for c in range(N_CORES):
        b, hg = c // 2, c % 2
        in_maps.append({
            "XqT": XqTs[b], "XkT": XkTs[b], "XvT": XvTs[b],
            "Wq": Wqs[hg], "Wk": Wks[hg], "Wv": Wvs[hg], "Wo": Wos[hg],
            "bq": bqs[hg], "bk": bks[hg], "bv": bvs[hg],
        })
    _last_in_maps = in_maps
    res = run_bass_kernel_spmd(nc, in_maps, list(range(N_CORES)))
    globals()['_last_res'] = res
    out = np.empty((B, S, D), dtype=np.float32)
    bO = np.asarray(b_O, dtype=np.float32)
    for b in range(B):
        out[b] = (np.asarray(res.results[2 * b]["OP"], dtype=np.float32)
                  + np.asarray(res.results[2 * b + 1]["OP"], dtype=np.float32)
                  + bO)
    return out


# revision 24
# speedup vs baseline: 1.1244x; 1.1244x over previous
"""Multi-head attention (B=4, S=2048, D=1024, H=16) on 8 trn2 NeuronCores.

Sharding: core c -> (batch b = c//2, head-group hg = c%2 of 8 heads).
Each core computes q/k/v projections for its 8 heads, attention, and a
partial output projection (its heads' contribution). Host sums the two
partials per batch and adds b_O.

Per-core device pipeline:
  1. projections: qT/kT [512,2048] (head-pair stacked on partitions),
     v-hat [128, 8, 65] in natural [s,c] layout (lhsT=XvT tile, rhs=Wv)
     with a ones column appended (softmax Z falls out of the PV matmul)
  2. per (head-pair, q-half): scoresT = kT.T @ qT (two heads row-packed,
     K=64), ACT exp(scale=1/8) -> PT, PV accumulates out_unT[65, q]
  3. tail: stage psum->SBUF (frees PSUM fast), recipZ, PE K=1 broadcast
     matmul, multiply -> attn_outT [512, 2048]
  4. output projection: attn_outT.T @ Wo -> partial [2048, 1024] fp32
"""
import sys

if '/opt/trn_rl_repo' not in sys.path:
    sys.path.insert(0, '/opt/trn_rl_repo')

import ml_dtypes
import numpy as np

import concourse.bass as bass
import concourse.tile as tile
from concourse import bacc, mybir
from concourse.bass_utils import run_bass_kernel_spmd

N_CORES = 8
B, S, D = 4, 2048, 1024
H = 16
DH = 64                 # head dim
HC = 8                  # heads per core
C = HC * DH             # per-core projection width = 512
F32 = mybir.dt.float32
F32R = mybir.dt.float32r
BF16 = mybir.dt.bfloat16

NKT = S // 128          # 16 s-tiles of 128
NM = C // 128           # 4 c-tiles (head pairs)
NDK = D // 128          # 8 contraction tiles for projections
SCALE = 1.0 / np.sqrt(DH)

# dtype config for the four matmul stages (BF16 or F32R)
PROJ_DT = BF16          # q/k/v projection inputs (XT, W)
QK_DT = BF16            # qT/kT tiles (scores matmul inputs)
PV_DT = BF16            # PT + v-hat (PV matmul inputs)
OUT_DT = BF16           # attn_outT + Wo (output projection inputs)


def round_fp32r(x):
    b = np.ascontiguousarray(x, dtype=np.float32).view(np.uint32)
    b = (b + 0x800) & np.uint32(0xFFFFF000)
    return b.view(np.float32)


def prep(x, dt):
    if dt == BF16:
        return np.ascontiguousarray(x).astype(ml_dtypes.bfloat16)
    return round_fp32r(x)


def build():
    nc = bacc.Bacc("TRN2", target_bir_lowering=False, debug=False,
                   num_devices=N_CORES)
    XqT = nc.dram_tensor("XqT", [D, S], PROJ_DT, kind="ExternalInput").ap()
    XkT = nc.dram_tensor("XkT", [D, S], PROJ_DT, kind="ExternalInput").ap()
    XvT = nc.dram_tensor("XvT", [D, S], PROJ_DT, kind="ExternalInput").ap()
    Wq = nc.dram_tensor("Wq", [D, C], PROJ_DT, kind="ExternalInput").ap()
    Wk = nc.dram_tensor("Wk", [D, C], PROJ_DT, kind="ExternalInput").ap()
    Wv = nc.dram_tensor("Wv", [D, C], PROJ_DT, kind="ExternalInput").ap()
    Wo = nc.dram_tensor("Wo", [C, D], OUT_DT, kind="ExternalInput").ap()
    bq = nc.dram_tensor("bq", [C], F32, kind="ExternalInput").ap()
    bk = nc.dram_tensor("bk", [C], F32, kind="ExternalInput").ap()
    bv = nc.dram_tensor("bv", [C], F32, kind="ExternalInput").ap()
    OP = nc.dram_tensor("OP", [S, D], F32, kind="ExternalOutput").ap()

    with tile.TileContext(nc) as tc:
        _build_body(nc, tc, XqT, XkT, XvT, Wq, Wk, Wv, Wo, bq, bk, bv, OP)
    nc.compile()
    return nc


def _build_body(nc, tc, XqT, XkT, XvT, Wq, Wk, Wv, Wo, bq, bk, bv, OP):
    from contextlib import ExitStack
    with ExitStack() as stack:
        consts = stack.enter_context(tc.tile_pool(name="consts", bufs=1))
        qkp = stack.enter_context(tc.tile_pool(name="qk", bufs=2 * NM))
        vhp = stack.enter_context(tc.tile_pool(name="vh", bufs=NKT))
        aop = stack.enter_context(tc.tile_pool(name="aout", bufs=NM))

        # constants
        ones_f32 = consts.tile([128, 1], F32)
        nc.vector.memset(ones_f32, 1.0)

        bias_t = consts.tile([128, 2 * NM], F32)
        for i, b_ in enumerate((bq, bk)):
            nc.sync.dma_start(
                out=bias_t[:, i * NM:(i + 1) * NM],
                in_=b_.rearrange("(m p) -> p m", p=128))
        bvb = consts.tile([128, C], F32)
        nc.gpsimd.dma_start(
            out=bvb,
            in_=bass.AP(tensor=bv.tensor, offset=0, ap=[[0, 128], [1, C]]))

        # ---------------- phase 1: projections ----------------
        qT = [None] * NM
        kT = [None] * NM
        vhat = [None] * NKT
        with ExitStack() as pstack:
            xtp = pstack.enter_context(tc.tile_pool(name="xt", bufs=16))
            wp = pstack.enter_context(tc.tile_pool(name="w", bufs=12))
            pjp = pstack.enter_context(
                tc.tile_pool(name="pj", bufs=3, space="PSUM"))

            for m in range(NM):
                qT[m] = qkp.tile([128, S], QK_DT, tag="qk", name=f"qTt{m}")
                kT[m] = qkp.tile([128, S], QK_DT, tag="qk", name=f"kTt{m}")

            # v in natural [s, c] layout: lhsT = XvT tile, rhs = Wv
            for half in range(2):
                xts = []
                ws = []
                for kt in range(NDK):
                    xt = xtp.tile([128, S // 2], PROJ_DT, tag="xt",
                                  name=f"xvt{half}_{kt}")
                    nc.sync.dma_start(
                        out=xt,
                        in_=XvT[kt * 128:(kt + 1) * 128,
                                half * (S // 2):(half + 1) * (S // 2)])
                    xts.append(xt)
                    w = wp.tile([128, C], PROJ_DT, tag="w",
                                name=f"wv{half}_{kt}")
                    nc.gpsimd.dma_start(
                        out=w, in_=Wv[kt * 128:(kt + 1) * 128, :])
                    ws.append(w)
                for stl in range(8):
                    st = half * 8 + stl
                    ps = pjp.tile([128, C], F32, tag="pj", name=f"vps{st}")
                    for kt in range(NDK):
                        nc.tensor.matmul(
                            ps,
                            xts[kt][:, stl * 128:(stl + 1) * 128],
                            ws[kt],
                            start=(kt == 0), stop=(kt == NDK - 1))
                    vh = vhp.tile([128, HC, DH + 1], PV_DT, tag="vh",
                                  name=f"vhat{st}")
                    with nc.allow_low_precision(reason="v epilogue"):
                        nc.vector.tensor_add(
                            vh[:, :, 0:DH],
                            ps.rearrange("p (h d) -> p h d", h=HC),
                            bvb.rearrange("p (h d) -> p h d", h=HC))
                        nc.vector.tensor_copy(
                            vh[:, :, DH], ones_f32.broadcast_to((128, HC)))
                    vhat[st] = vh

            def projection(XT, W, bcol, outs):
                for half in range(2):
                    xts = []
                    ws = []
                    for kt in range(NDK):
                        xt = xtp.tile([128, S // 2], PROJ_DT, tag="xt")
                        nc.sync.dma_start(
                            out=xt,
                            in_=XT[kt * 128:(kt + 1) * 128,
                                   half * (S // 2):(half + 1) * (S // 2)])
                        xts.append(xt)
                        w = wp.tile([128, C], PROJ_DT, tag="w")
                        nc.gpsimd.dma_start(
                            out=w, in_=W[kt * 128:(kt + 1) * 128, :])
                        ws.append(w)
                    for m in range(NM):
                        for sc in range(2):
                            ps = pjp.tile([128, 512], F32, tag="pj")
                            for kt in range(NDK):
                                nc.tensor.matmul(
                                    ps,
                                    ws[kt][:, m * 128:(m + 1) * 128],
                                    xts[kt][:, sc * 512:(sc + 1) * 512],
                                    start=(kt == 0), stop=(kt == NDK - 1))
                            s0 = half * (S // 2) + sc * 512
                            with nc.allow_low_precision(reason="proj epi"):
                                nc.scalar.activation(
                                    out=outs[m][:, s0:s0 + 512], in_=ps,
                                    func=mybir.ActivationFunctionType.Identity,
                                    bias=bias_t[:, bcol + m:bcol + m + 1],
                                    scale=1.0)

            projection(XkT, Wk, NM, kT)
            projection(XqT, Wq, 0, qT)

        # ---------------- phase 2: attention ----------------
        attn_outT = [None] * NM
        for m in range(NM):
            attn_outT[m] = aop.tile([128, S], OUT_DT, tag="aout",
                                    name=f"aoutT{m}")

        with ExitStack() as astack:
            ptp = astack.enter_context(tc.tile_pool(name="pt", bufs=8))
            stg = astack.enter_context(tc.tile_pool(name="stg", bufs=6))
            nrm = astack.enter_context(tc.tile_pool(name="nrm", bufs=6))
            wop = astack.enter_context(tc.tile_pool(name="wo", bufs=NM))
            oap = astack.enter_context(tc.tile_pool(name="oacc", bufs=32))
            sp = astack.enter_context(
                tc.tile_pool(name="sps", bufs=2, space="PSUM"))
            pvp = astack.enter_context(
                tc.tile_pool(name="pv", bufs=2, space="PSUM"))
            opp = astack.enter_context(
                tc.tile_pool(name="op", bufs=2, space="PSUM"))

            wo_tiles = []
            for m in range(NM):
                w = wop.tile([128, D], OUT_DT, tag="wo", name=f"wo{m}")
                nc.sync.dma_start(out=w, in_=Wo[m * 128:(m + 1) * 128, :])
                wo_tiles.append(w)
            out_acc = [[None] * 2 for _ in range(NKT)]

            def outproj_piece(hp, st):
                for oc in range(2):
                    ps = opp.tile([128, 512], F32, tag="op",
                                  name=f"ops{hp}_{st}_{oc}")
                    nc.tensor.matmul(
                        ps,
                        attn_outT[hp][:, st * 128:(st + 1) * 128],
                        wo_tiles[hp][:, oc * 512:(oc + 1) * 512],
                        start=True, stop=True)
                    if hp == 0:
                        oa = oap.tile([128, 512], F32, tag="oacc",
                                      name=f"oacc{st}_{oc}")
                        out_acc[st][oc] = oa
                        nc.vector.tensor_copy(oa, ps)
                    else:
                        oa = out_acc[st][oc]
                        nc.vector.tensor_add(oa, oa, ps)
                    if hp == NM - 1:
                        nc.sync.dma_start(
                            out=OP[st * 128:(st + 1) * 128,
                                   oc * 512:(oc + 1) * 512],
                            in_=oa)

            # deferred tail/outproj pieces, drained at fixed kt slots of
            # the NEXT block so slow DVE work never head-of-line blocks PE
            deferred = []

            def make_tail_pieces(hp, qc, pvA, pvB):
                q0 = qc * 512
                sts = [None, None]
                rzs = [None, None]
                bcs = [None, None]

                def stage(hh):
                    acc = pvA if hh == 0 else pvB
                    st_t = stg.tile([DH + 1, 512], F32, tag="stg",
                                    name=f"stg{hp}_{qc}_{hh}")
                    nc.vector.tensor_copy(st_t, acc)
                    sts[hh] = st_t

                def recip(hh):
                    rz = nrm.tile([1, 512], F32, tag="rz",
                                  name=f"rz{hp}_{qc}_{hh}")
                    nc.vector.reciprocal(out=rz, in_=sts[hh][DH:DH + 1, :])
                    rzs[hh] = rz

                def bcast(hh):
                    bc = nrm.tile([DH, 512], F32, tag="bc",
                                  name=f"bc{hp}_{qc}_{hh}")
                    nc.gpsimd.partition_broadcast(bc, rzs[hh])
                    bcs[hh] = bc

                def mul(hh):
                    dlo = hh * DH
                    with nc.allow_low_precision(reason="attn_outT"):
                        nc.vector.tensor_mul(
                            attn_outT[hp][dlo:dlo + DH, q0:q0 + 512],
                            sts[hh][0:DH, :], bcs[hh])

                # stages run NOW (free the PSUM accumulators quickly)
                stage(0)
                stage(1)
                return [
                    lambda: recip(0),
                    lambda: recip(1),
                    lambda: bcast(0),
                    lambda: bcast(1),
                    lambda: mul(0),
                    lambda: mul(1),
                    lambda: outproj_piece(hp, qc * 4 + 0),
                    lambda: outproj_piece(hp, qc * 4 + 1),
                    lambda: outproj_piece(hp, qc * 4 + 2),
                    lambda: outproj_piece(hp, qc * 4 + 3),
                ]

            # kt slots at which deferred pieces fire (10 pieces)
            SLOTS = {0: 0, 1: 1, 3: 2, 4: 3, 6: 4, 7: 5,
                     9: 6, 11: 7, 13: 8, 15: 9}

            for hp in range(NM):
                for qc in range(4):
                    q0 = qc * 512
                    pvA = pvp.tile([DH + 1, 512], F32, tag="pv",
                                   name=f"pvA{hp}_{qc}")
                    pvB = pvp.tile([DH + 1, 512], F32, tag="pv",
                                   name=f"pvB{hp}_{qc}")
                    for kt in range(NKT):
                        sps = sp.tile([128, 1024], F32, tag="sps")
                        for hh in range(2):
                            dlo = hh * DH
                            nc.tensor.matmul(
                                sps[:, hh * 512:(hh + 1) * 512],
                                kT[hp][dlo:dlo + DH,
                                       kt * 128:(kt + 1) * 128],
                                qT[hp][dlo:dlo + DH, q0:q0 + 512],
                                start=True, stop=True)
                        pt = ptp.tile([128, 1024], PV_DT, tag="pt")
                        nc.scalar.activation(
                            out=pt, in_=sps,
                            func=mybir.ActivationFunctionType.Exp,
                            scale=float(SCALE))
                        nc.tensor.matmul(
                            pvA, vhat[kt][:, 2 * hp, :], pt[:, 0:512],
                            start=(kt == 0), stop=(kt == NKT - 1))
                        nc.tensor.matmul(
                            pvB, vhat[kt][:, 2 * hp + 1, :], pt[:, 512:1024],
                            start=(kt == 0), stop=(kt == NKT - 1))
                        if kt in SLOTS and deferred:
                            deferred[SLOTS[kt]]()
                    deferred = make_tail_pieces(hp, qc, pvA, pvB)

            # flush the last block's pieces
            for piece in deferred:
                piece()


_NC_CACHE = None
_last_in_maps = None


def _get_nc():
    global _NC_CACHE
    if _NC_CACHE is None:
        _NC_CACHE = build()
    return _NC_CACHE


def kernel(Q, K, V, W_Q, b_Q, W_K, b_K, W_V, b_V, W_O, b_O):
    global _last_in_maps
    Q = np.asarray(Q, dtype=np.float32)
    K = np.asarray(K, dtype=np.float32)
    V = np.asarray(V, dtype=np.float32)
    nc = _get_nc()

    XqTs = [prep(Q[b].T, PROJ_DT) for b in range(B)]
    XkTs = [prep(K[b].T, PROJ_DT) for b in range(B)]
    XvTs = [prep(V[b].T, PROJ_DT) for b in range(B)]
    Wqs = [prep(np.asarray(W_Q)[:, hg * C:(hg + 1) * C], PROJ_DT)
           for hg in range(2)]
    Wks = [prep(np.asarray(W_K)[:, hg * C:(hg + 1) * C], PROJ_DT)
           for hg in range(2)]
    Wvs = [prep(np.asarray(W_V)[:, hg * C:(hg + 1) * C], PROJ_DT)
           for hg in range(2)]
    Wos = [prep(np.asarray(W_O)[hg * C:(hg + 1) * C, :], OUT_DT)
           for hg in range(2)]
    bqs = [np.ascontiguousarray(np.asarray(b_Q, dtype=np.float32)[hg * C:(hg + 1) * C])
           for hg in range(2)]
    bks = [np.ascontiguousarray(np.asarray(b_K, dtype=np.float32)[hg * C:(hg + 1) * C])
           for hg in range(2)]
    bvs = [np.ascontiguousarray(np.asarray(b_V, dtype=np.float32)[hg * C:(hg + 1) * C])
           for hg in range(2)]

    in_maps = []
    for c in range(N_CORES):
        b, hg = c // 2, c % 2
        in_maps.append({
            "XqT": XqTs[b], "XkT": XkTs[b], "XvT": XvTs[b],
            "Wq": Wqs[hg], "Wk": Wks[hg], "Wv": Wvs[hg], "Wo": Wos[hg],
            "bq": bqs[hg], "bk": bks[hg], "bv": bvs[hg],
        })
    _last_in_maps = in_maps
    res = run_bass_kernel_spmd(nc, in_maps, list(range(N_CORES)))
    out = np.empty((B, S, D), dtype=np.float32)
    bO = np.asarray(b_O, dtype=np.float32)
    for b in range(B):
        out[b] = res.results[2 * b]["OP"] + res.results[2 * b + 1]["OP"] + bO
    return out



# revision 25
# speedup vs baseline: 1.2295x; 1.0934x over previous
"""Multi-head attention (B=4, S=2048, D=1024, H=16) on 8 trn2 NeuronCores.

Sharding: core c -> (batch b = c//2, head-group hg = c%2 of 8 heads).
Each core computes q/k/v projections for its 8 heads, attention, and a
partial output projection (its heads' contribution). Host sums the two
partials per batch and adds b_O.

Per-core device pipeline:
  1. projections: qT/kT [512,2048] (head-pair stacked on partitions),
     v-hat [128, 8, 65] in natural [s,c] layout (lhsT=XvT tile, rhs=Wv)
     with a ones column appended (softmax Z falls out of the PV matmul)
  2. per (head-pair, q-half): scoresT = kT.T @ qT (two heads row-packed,
     K=64), ACT exp(scale=1/8) -> PT, PV accumulates out_unT[65, q]
  3. tail: stage psum->SBUF (frees PSUM fast), recipZ, PE K=1 broadcast
     matmul, multiply -> attn_outT [512, 2048]
  4. output projection: attn_outT.T @ Wo -> partial [2048, 1024] fp32
"""
import sys

if '/opt/trn_rl_repo' not in sys.path:
    sys.path.insert(0, '/opt/trn_rl_repo')

import ml_dtypes
import numpy as np

import concourse.bass as bass
import concourse.tile as tile
from concourse import bacc, mybir
from concourse.bass_utils import run_bass_kernel_spmd

N_CORES = 8
B, S, D = 4, 2048, 1024
H = 16
DH = 64                 # head dim
HC = 8                  # heads per core
C = HC * DH             # per-core projection width = 512
F32 = mybir.dt.float32
F32R = mybir.dt.float32r
BF16 = mybir.dt.bfloat16

NKT = S // 128          # 16 s-tiles of 128
NM = C // 128           # 4 c-tiles (head pairs)
NDK = D // 128          # 8 contraction tiles for projections
SCALE = 1.0 / np.sqrt(DH)

# dtype config for the four matmul stages (BF16 or F32R)
PROJ_DT = BF16          # q/k/v projection inputs (XT, W)
QK_DT = BF16            # qT/kT tiles (scores matmul inputs)
PV_DT = BF16            # PT + v-hat (PV matmul inputs)
OUT_DT = BF16           # attn_outT + Wo (output projection inputs)


def round_fp32r(x):
    b = np.ascontiguousarray(x, dtype=np.float32).view(np.uint32)
    b = (b + 0x800) & np.uint32(0xFFFFF000)
    return b.view(np.float32)


def prep(x, dt):
    if dt == BF16:
        return np.ascontiguousarray(x).astype(ml_dtypes.bfloat16)
    return round_fp32r(x)


def build():
    nc = bacc.Bacc("TRN2", target_bir_lowering=False, debug=False,
                   num_devices=N_CORES)
    XqT = nc.dram_tensor("XqT", [D, S], PROJ_DT, kind="ExternalInput").ap()
    XkT = nc.dram_tensor("XkT", [D, S], PROJ_DT, kind="ExternalInput").ap()
    XvT = nc.dram_tensor("XvT", [D, S], PROJ_DT, kind="ExternalInput").ap()
    Wq = nc.dram_tensor("Wq", [D, C], PROJ_DT, kind="ExternalInput").ap()
    Wk = nc.dram_tensor("Wk", [D, C], PROJ_DT, kind="ExternalInput").ap()
    Wv = nc.dram_tensor("Wv", [D, C], PROJ_DT, kind="ExternalInput").ap()
    Wo = nc.dram_tensor("Wo", [C, D], OUT_DT, kind="ExternalInput").ap()
    bq = nc.dram_tensor("bq", [C], F32, kind="ExternalInput").ap()
    bk = nc.dram_tensor("bk", [C], F32, kind="ExternalInput").ap()
    bv = nc.dram_tensor("bv", [C], F32, kind="ExternalInput").ap()
    OP = nc.dram_tensor("OP", [S, D], F32, kind="ExternalOutput").ap()

    with tile.TileContext(nc) as tc:
        _build_body(nc, tc, XqT, XkT, XvT, Wq, Wk, Wv, Wo, bq, bk, bv, OP)
    nc.compile()
    return nc


def _build_body(nc, tc, XqT, XkT, XvT, Wq, Wk, Wv, Wo, bq, bk, bv, OP):
    from contextlib import ExitStack
    with ExitStack() as stack:
        consts = stack.enter_context(tc.tile_pool(name="consts", bufs=1))
        qkp = stack.enter_context(tc.tile_pool(name="qk", bufs=2 * NM))
        vhp = stack.enter_context(tc.tile_pool(name="vh", bufs=NKT))
        aop = stack.enter_context(tc.tile_pool(name="aout", bufs=NM))

        # constants
        ones_f32 = consts.tile([128, 1], F32)
        nc.vector.memset(ones_f32, 1.0)
        ones_row = consts.tile([1, DH], F32R)
        with nc.allow_low_precision(reason="exact ones to fp32r"):
            nc.vector.tensor_copy(
                ones_row, ones_f32[0:1, :].broadcast_to((1, DH)))

        bias_t = consts.tile([128, 2 * NM], F32)
        for i, b_ in enumerate((bq, bk)):
            nc.sync.dma_start(
                out=bias_t[:, i * NM:(i + 1) * NM],
                in_=b_.rearrange("(m p) -> p m", p=128))
        bvb = consts.tile([128, C], F32)
        nc.gpsimd.dma_start(
            out=bvb,
            in_=bass.AP(tensor=bv.tensor, offset=0, ap=[[0, 128], [1, C]]))

        # ---------------- phase 1: projections ----------------
        qT = [None] * NM
        kT = [None] * NM
        vhat = [None] * NKT
        with ExitStack() as pstack:
            xtp = pstack.enter_context(tc.tile_pool(name="xt", bufs=16))
            wp = pstack.enter_context(tc.tile_pool(name="w", bufs=12))
            pjp = pstack.enter_context(
                tc.tile_pool(name="pj", bufs=3, space="PSUM"))

            for m in range(NM):
                qT[m] = qkp.tile([128, S], QK_DT, tag="qk", name=f"qTt{m}")
                kT[m] = qkp.tile([128, S], QK_DT, tag="qk", name=f"kTt{m}")

            # v in natural [s, c] layout: lhsT = XvT tile, rhs = Wv
            wvs = []
            for kt in range(NDK):
                w = wp.tile([128, C], PROJ_DT, tag="w", name=f"wv{kt}")
                nc.gpsimd.dma_start(out=w, in_=Wv[kt * 128:(kt + 1) * 128, :])
                wvs.append(w)
            for half in range(2):
                xts = []
                for kt in range(NDK):
                    xt = xtp.tile([128, S // 2], PROJ_DT, tag="xt",
                                  name=f"xvt{half}_{kt}")
                    eng = nc.sync if kt % 2 == 0 else nc.scalar
                    eng.dma_start(
                        out=xt,
                        in_=XvT[kt * 128:(kt + 1) * 128,
                                half * (S // 2):(half + 1) * (S // 2)])
                    xts.append(xt)
                ws = wvs
                for stl in range(8):
                    st = half * 8 + stl
                    ps = pjp.tile([128, C], F32, tag="pj", name=f"vps{st}")
                    for kt in range(NDK):
                        nc.tensor.matmul(
                            ps,
                            xts[kt][:, stl * 128:(stl + 1) * 128],
                            ws[kt],
                            start=(kt == 0), stop=(kt == NDK - 1))
                    vh = vhp.tile([128, HC, DH + 1], PV_DT, tag="vh",
                                  name=f"vhat{st}")
                    with nc.allow_low_precision(reason="v epilogue"):
                        nc.vector.tensor_add(
                            vh[:, :, 0:DH],
                            ps.rearrange("p (h d) -> p h d", h=HC),
                            bvb.rearrange("p (h d) -> p h d", h=HC))
                        nc.vector.tensor_copy(
                            vh[:, :, DH], ones_f32.broadcast_to((128, HC)))
                    vhat[st] = vh

            def projection(XT, W, bcol, outs):
                wts = []
                for kt in range(NDK):
                    w = wp.tile([128, C], PROJ_DT, tag="w")
                    nc.gpsimd.dma_start(
                        out=w, in_=W[kt * 128:(kt + 1) * 128, :])
                    wts.append(w)
                for half in range(2):
                    xts = []
                    for kt in range(NDK):
                        xt = xtp.tile([128, S // 2], PROJ_DT, tag="xt")
                        eng = nc.sync if kt % 2 == 0 else nc.scalar
                        eng.dma_start(
                            out=xt,
                            in_=XT[kt * 128:(kt + 1) * 128,
                                   half * (S // 2):(half + 1) * (S // 2)])
                        xts.append(xt)
                    ws = wts
                    for m in range(NM):
                        for sc in range(2):
                            ps = pjp.tile([128, 512], F32, tag="pj")
                            for kt in range(NDK):
                                nc.tensor.matmul(
                                    ps,
                                    ws[kt][:, m * 128:(m + 1) * 128],
                                    xts[kt][:, sc * 512:(sc + 1) * 512],
                                    start=(kt == 0), stop=(kt == NDK - 1))
                            s0 = half * (S // 2) + sc * 512
                            with nc.allow_low_precision(reason="proj epi"):
                                nc.scalar.activation(
                                    out=outs[m][:, s0:s0 + 512], in_=ps,
                                    func=mybir.ActivationFunctionType.Identity,
                                    bias=bias_t[:, bcol + m:bcol + m + 1],
                                    scale=1.0)

            projection(XkT, Wk, NM, kT)
            projection(XqT, Wq, 0, qT)

        # ---------------- phase 2: attention ----------------
        attn_outT = [None] * NM
        for m in range(NM):
            attn_outT[m] = aop.tile([128, S], OUT_DT, tag="aout",
                                    name=f"aoutT{m}")

        with ExitStack() as astack:
            ptp = astack.enter_context(tc.tile_pool(name="pt", bufs=8))
            stg = astack.enter_context(tc.tile_pool(name="stg", bufs=6))
            nrm = astack.enter_context(tc.tile_pool(name="nrm", bufs=6))
            wop = astack.enter_context(tc.tile_pool(name="wo", bufs=NM))
            oap = astack.enter_context(tc.tile_pool(name="oacc", bufs=32))
            sp = astack.enter_context(
                tc.tile_pool(name="sps", bufs=2, space="PSUM"))
            pvp = astack.enter_context(
                tc.tile_pool(name="pv", bufs=2, space="PSUM"))
            opp = astack.enter_context(
                tc.tile_pool(name="op", bufs=2, space="PSUM"))

            wo_tiles = []
            for m in range(NM):
                w = wop.tile([128, D], OUT_DT, tag="wo", name=f"wo{m}")
                nc.sync.dma_start(out=w, in_=Wo[m * 128:(m + 1) * 128, :])
                wo_tiles.append(w)
            out_acc = [[None] * 2 for _ in range(NKT)]

            def outproj_piece(hp, st):
                for oc in range(2):
                    ps = opp.tile([128, 512], F32, tag="op",
                                  name=f"ops{hp}_{st}_{oc}")
                    nc.tensor.matmul(
                        ps,
                        attn_outT[hp][:, st * 128:(st + 1) * 128],
                        wo_tiles[hp][:, oc * 512:(oc + 1) * 512],
                        start=True, stop=True)
                    if hp == 0:
                        oa = oap.tile([128, 512], F32, tag="oacc",
                                      name=f"oacc{st}_{oc}")
                        out_acc[st][oc] = oa
                        nc.vector.tensor_copy(oa, ps)
                    else:
                        oa = out_acc[st][oc]
                        nc.vector.tensor_add(oa, oa, ps)
                    if hp == NM - 1:
                        nc.sync.dma_start(
                            out=OP[st * 128:(st + 1) * 128,
                                   oc * 512:(oc + 1) * 512],
                            in_=oa)

            # deferred tail/outproj pieces, drained at fixed kt slots of
            # the NEXT block so slow DVE work never head-of-line blocks PE
            deferred = []

            def make_tail_pieces(hp, qc, pvA, pvB):
                q0 = qc * 512
                sts = [None, None]
                rzs = [None, None]
                bcs = [None, None]

                def stage(hh):
                    acc = pvA if hh == 0 else pvB
                    st_t = stg.tile([DH + 1, 512], F32R, tag="stg",
                                    name=f"stg{hp}_{qc}_{hh}")
                    with nc.allow_low_precision(reason="stage"):
                        nc.vector.tensor_copy(st_t, acc)
                    sts[hh] = st_t

                def recip(hh):
                    rz = nrm.tile([1, 512], F32R, tag="rz",
                                  name=f"rz{hp}_{qc}_{hh}")
                    with nc.allow_low_precision(reason="recipZ"):
                        nc.vector.reciprocal(out=rz, in_=sts[hh][DH:DH + 1, :])
                    rzs[hh] = rz

                def bcast(hh):
                    bc_ps = opp.tile([DH, 512], F32, tag="op",
                                     name=f"bcp{hp}_{qc}_{hh}")
                    nc.tensor.matmul(bc_ps, ones_row, rzs[hh],
                                     start=True, stop=True)
                    bc = nrm.tile([DH, 512], F32R, tag="bc",
                                  name=f"bc{hp}_{qc}_{hh}")
                    with nc.allow_low_precision(reason="bc"):
                        nc.vector.tensor_copy(bc, bc_ps)
                    bcs[hh] = bc

                def mul(hh):
                    dlo = hh * DH
                    with nc.allow_low_precision(reason="attn_outT"):
                        nc.vector.tensor_mul(
                            attn_outT[hp][dlo:dlo + DH, q0:q0 + 512],
                            sts[hh][0:DH, :], bcs[hh])

                # stages run NOW (free the PSUM accumulators quickly)
                stage(0)
                stage(1)
                return [
                    lambda: recip(0),
                    lambda: recip(1),
                    lambda: bcast(0),
                    lambda: bcast(1),
                    lambda: mul(0),
                    lambda: mul(1),
                    lambda: outproj_piece(hp, qc * 4 + 0),
                    lambda: outproj_piece(hp, qc * 4 + 1),
                    lambda: outproj_piece(hp, qc * 4 + 2),
                    lambda: outproj_piece(hp, qc * 4 + 3),
                ]

            # kt slots at which deferred pieces fire (10 pieces); the slow
            # [1,512] reciprocals run first with ~5 kt of slack before the
            # PE broadcast pieces consume them
            SLOTS = {0: 0, 1: 1, 5: 2, 8: 3, 9: 4, 11: 5,
                     12: 6, 13: 7, 14: 8, 15: 9}

            for hp in range(NM):
                for qc in range(4):
                    q0 = qc * 512
                    pvA = pvp.tile([DH + 1, 512], F32, tag="pv",
                                   name=f"pvA{hp}_{qc}")
                    pvB = pvp.tile([DH + 1, 512], F32, tag="pv",
                                   name=f"pvB{hp}_{qc}")
                    for kt in range(NKT):
                        sps = sp.tile([128, 1024], F32, tag="sps")
                        for hh in range(2):
                            dlo = hh * DH
                            nc.tensor.matmul(
                                sps[:, hh * 512:(hh + 1) * 512],
                                kT[hp][dlo:dlo + DH,
                                       kt * 128:(kt + 1) * 128],
                                qT[hp][dlo:dlo + DH, q0:q0 + 512],
                                start=True, stop=True)
                        pt = ptp.tile([128, 1024], PV_DT, tag="pt")
                        nc.scalar.activation(
                            out=pt, in_=sps,
                            func=mybir.ActivationFunctionType.Exp,
                            scale=float(SCALE))
                        nc.tensor.matmul(
                            pvA, vhat[kt][:, 2 * hp, :], pt[:, 0:512],
                            start=(kt == 0), stop=(kt == NKT - 1))
                        nc.tensor.matmul(
                            pvB, vhat[kt][:, 2 * hp + 1, :], pt[:, 512:1024],
                            start=(kt == 0), stop=(kt == NKT - 1))
                        if kt in SLOTS and deferred:
                            deferred[SLOTS[kt]]()
                    deferred = make_tail_pieces(hp, qc, pvA, pvB)

            # flush the last block's pieces
            for piece in deferred:
                piece()


_NC_CACHE = None
_last_in_maps = None


def _get_nc():
    global _NC_CACHE
    if _NC_CACHE is None:
        _NC_CACHE = build()
    return _NC_CACHE


def kernel(Q, K, V, W_Q, b_Q, W_K, b_K, W_V, b_V, W_O, b_O):
    global _last_in_maps
    Q = np.asarray(Q, dtype=np.float32)
    K = np.asarray(K, dtype=np.float32)
    V = np.asarray(V, dtype=np.float32)
    nc = _get_nc()

    XqTs = [prep(Q[b].T, PROJ_DT) for b in range(B)]
    XkTs = [prep(K[b].T, PROJ_DT) for b in range(B)]
    XvTs = [prep(V[b].T, PROJ_DT) for b in range(B)]
    Wqs = [prep(np.asarray(W_Q)[:, hg * C:(hg + 1) * C], PROJ_DT)
           for hg in range(2)]
    Wks = [prep(np.asarray(W_K)[:, hg * C:(hg + 1) * C], PROJ_DT)
           for hg in range(2)]
    Wvs = [prep(np.asarray(W_V)[:, hg * C:(hg + 1) * C], PROJ_DT)
           for hg in range(2)]
    Wos = [prep(np.asarray(W_O)[hg * C:(hg + 1) * C, :], OUT_DT)
           for hg in range(2)]
    bqs = [np.ascontiguousarray(np.asarray(b_Q, dtype=np.float32)[hg * C:(hg + 1) * C])
           for hg in range(2)]
    bks = [np.ascontiguousarray(np.asarray(b_K, dtype=np.float32)[hg * C:(hg + 1) * C])
           for hg in range(2)]
    bvs = [np.ascontiguousarray(np.asarray(b_V, dtype=np.float32)[hg * C:(hg + 1) * C])
           for hg in range(2)]

    in_maps = []
    for c in range(N_CORES):
        b, hg = c // 2, c % 2
        in_maps.append({
            "XqT": XqTs[b], "XkT": XkTs[b], "XvT": XvTs[b],
            "Wq": Wqs[hg], "Wk": Wks[hg], "Wv": Wvs[hg], "Wo": Wos[hg],
            "bq": bqs[hg], "bk": bks[hg], "bv": bvs[hg],
        })
    _last_in_maps = in_maps
    res = run_bass_kernel_spmd(nc, in_maps, list(range(N_CORES)))
    out = np.empty((B, S, D), dtype=np.float32)
    bO = np.asarray(b_O, dtype=np.float32)
    for b in range(B):
        out[b] = res.results[2 * b]["OP"] + res.results[2 * b + 1]["OP"] + bO
    return out

